# revision 8
# baseline (speedup 1.0000x reference)
"""Trainium2 Bass kernel for an additive-attention (GAT-style) head.

Reference math (N=8192, EMB=256, NHID=64, alpha=0.2):
    h      = input @ W                               [N, 64]
    s_src  = h @ a[:64];  s_dst = h @ a[64:]         [N]
    e      = leaky_relu(s_src[:,None] + s_dst[None,:], 0.2)
    att    = softmax(where(adj > 0, e, -9e15), axis=1)
    out    = att @ h                                 [N, 64]

w-gauge restructuring (row i divided by exp(s_src_i), column factor v folded
into the payload):
    p'_ij  = A_ij * max(r_i * w_j, 1)     r = exp((a-1)s_src), w = exp((a-1)s_dst)
    payV_j = exp(s_dst_j) * (h_j | 1)
    out_i  = (p'_i: @ payV[:, 0:64]) / (p'_i: @ payV[:, 64])

Distribution: 1-D row partition of N across 8 cores (1024 rows each); every
core redundantly computes the full payload from a replicated bf16 input^T.

Engine split (the 2-elementwise-op N^2 pass is the fundamental wall, so tiles
are fanned across three engines; adj SBUF-write bytes are the DMA wall, so a
slice of adj ships as fp8 for GpSimd which eats fp8 at full speed):
  x-tiles: DVE ts kap'=max(r*w,1) + fused DVE tt (p' = kap' * A_bf16)
  y-tiles: ScalarE act krelu=relu(r*w-1) + fused DVE tt (m = krelu * A) and an
           extra PE matmul pair accumulating payV^T @ A  (m + A = p')
  g-tiles: DVE ts kap' + GpSimd tt (p' = kap' * A_fp8)
Payload: 2 ldweights (W|W@a_dst) + 32 big matmuls -> h^T in PSUM -> bf16 SBUF
-> PE transposes (ident stationary) -> ScalarE v-scaled copies into payV.
"""

import sys

sys.path.insert(0, "/opt/trn_rl_repo")

import ml_dtypes
import numpy as np
from contextlib import ExitStack

import concourse.bass as bass
import concourse.mybir as mybir
import concourse.tile as tile

N = 8192
EMB = 256
NHID = 64
ALPHA = 0.2
NCORES = 8
NLOC = N // NCORES          # 1024 rows per core
NT = N // 128               # 64 j-tiles
NG = 8                      # tile groups of 8
NHE = NHID + 1              # h plus denominator column
FP32 = mybir.dt.float32
BF16 = mybir.dt.bfloat16
FP8 = mybir.dt.float8e4

AX = mybir.AxisListType
ALU = mybir.AluOpType
ACTF = mybir.ActivationFunctionType

# form assignment within each group of 8 j-tiles
N_BF = 8                    # slots 0..7 carried as bf16 (x/y forms)
N_F8 = 0                    # no fp8/GpSimd slots (GpSimd-DVE contention loses)
Y_GROUPS = tuple(range(8))  # groups whose Y_SLOTS use the ScalarE relu form
Y_SLOTS = (4, 5)


def _forms():
    out = {}
    for gr in range(NG):
        for slot in range(8):
            if slot >= N_BF:
                f = "g"
            elif slot in Y_SLOTS and gr in Y_GROUPS:
                f = "y"
            else:
                f = "x"
            out[(gr, slot)] = f
    return out


FORM = _forms()


class WaitSplitTileContext(tile.TileContext):
    """walrus' S3_LW (ldweights/matmul) struct accepts only ONE sync-wait
    command; Tile can emit matmuls with several.  Hoist the excess waits onto
    standalone InstEventSemaphore instructions on the same engine, inserted
    immediately before the matmul in the final scheduled order."""

    _NO_SPLIT_TYPES = (
        mybir.InstDrain,
        mybir.InstEventSemaphore,
    )

    def _add_instruction(self, inst):
        si = getattr(inst, "sync_info", None)
        if (
            si is not None
            and si.on_wait
            and len(si.on_wait) > 1
            and not isinstance(inst, self._NO_SPLIT_TYPES)
        ):
            waits = list(si.on_wait)
            for i, w in enumerate(waits[:-1]):
                ev = mybir.InstEventSemaphore(
                    name=f"{inst.name}-wsplit{i}",
                    engine=inst.engine,
                    ins=[],
                    outs=[],
                    sync_info=mybir.SyncInfo(on_wait=[w], on_update=[]),
                    bass_nofuse=True,
                )
                super()._add_instruction(ev)
            inst.sync_info = mybir.SyncInfo(
                on_wait=[waits[-1]], on_update=list(si.on_update)
            )
        super()._add_instruction(inst)

    def _drain_and_barrier(self, tick_clock, wait_clock):
        # The stock version attaches every engine's final tick as waits on ONE
        # drain -- over walrus' per-instruction limit.  Compute the waits on a
        # probe instruction, emit them as single-wait EventSemaphores on the
        # sync queue, then a clean drain.
        from concourse.vector_clock import ScopedClock

        probe = mybir.InstEventSemaphore(
            name=f"drain-wsplit-probe-{self.nc.next_id()}",
            engine=mybir.EngineType.SP,
            ins=[],
            outs=[],
            sync_info=None,
            bass_nofuse=True,
        )
        wait_clock.add_sem_waits(probe, ScopedClock({None: tick_clock.global_clock}))
        waits = list(probe.sync_info.on_wait) if probe.sync_info else []
        for i, w in enumerate(waits):
            ev = mybir.InstEventSemaphore(
                name=f"drain-wsplit{i}-{self.nc.next_id()}",
                engine=mybir.EngineType.SP,
                ins=[],
                outs=[],
                sync_info=mybir.SyncInfo(on_wait=[w], on_update=[]),
                bass_nofuse=True,
            )
            self._add_instruction(ev)
        self.nc.sync.drain()

        self.nc.all_engine_barrier()
        assert self.sems is not None
        popped = self.nc._tile_sem_poison_stack.pop()
        assert popped is self._sem_poison
        self.nc.clear_and_free_semaphores(list(self.sems.allocated().values()))
        self.nc.all_engine_barrier()


def build_kernel() -> bass.Bass:
    nc = bass.Bass(num_devices=NCORES)

    adjB_p = nc.declare_dram_parameter("adjB", [NG, 128, N_BF * NLOC], BF16, isOutput=False)
    adjF_p = nc.declare_dram_parameter("adjF", [NG, 128, N_F8 * NLOC], FP8, isOutput=False) if N_F8 else None
    inTl_p = nc.declare_dram_parameter("inTl", [EMB, NLOC], BF16, isOutput=False)
    inTf_p = nc.declare_dram_parameter("inTf", [EMB, N], BF16, isOutput=False)
    W_p = nc.declare_dram_parameter("W", [EMB, NHID], FP32, isOutput=False)
    WT_p = nc.declare_dram_parameter("WT", [NHID, EMB], FP32, isOutput=False)
    a_p = nc.declare_dram_parameter("a", [2 * NHID], FP32, isOutput=False)
    ident_p = nc.declare_dram_parameter("ident", [128, 128], FP32, isOutput=False)
    identB_p = nc.declare_dram_parameter("identB", [128, 128], BF16, isOutput=False)
    # [p, ic, e] layout == out_sb SBUF layout; host untangles (pure transpose)
    out_p = nc.declare_dram_parameter("out", [128, 8 * NHID], FP32, isOutput=True)

    with WaitSplitTileContext(nc) as tc, ExitStack() as ctx:
        const = ctx.enter_context(tc.tile_pool(name="const", bufs=1))
        ps_scr = ctx.enter_context(
            tc.tile_pool(name="ps_scr", bufs=2, space=bass.MemorySpace.PSUM)
        )
        ps_h = ctx.enter_context(
            tc.tile_pool(name="ps_h", bufs=1, space=bass.MemorySpace.PSUM)
        )
        ps_t4 = ctx.enter_context(
            tc.tile_pool(name="ps_t4", bufs=2, space=bass.MemorySpace.PSUM)
        )
        ps_acc = ctx.enter_context(
            tc.tile_pool(name="ps_acc", bufs=1, space=bass.MemorySpace.PSUM)
        )
        bf_pool = ctx.enter_context(tc.tile_pool(name="bfp", bufs=3))
        f8_pool = ctx.enter_context(tc.tile_pool(name="f8p", bufs=3))
        kap_pool = ctx.enter_context(tc.tile_pool(name="kapp", bufs=2))
        gkap_pool = ctx.enter_context(tc.tile_pool(name="gkapp", bufs=2))
        p_pool = ctx.enter_context(tc.tile_pool(name="pp", bufs=2))
        pg_pool = ctx.enter_context(tc.tile_pool(name="pgp", bufs=2))
        small = ctx.enter_context(tc.tile_pool(name="small", bufs=2))

        # ---- constant / preamble tiles ----
        inTl = [const.tile([128, NLOC], BF16, tag=f"inTl{k}", name=f"inTl{k}") for k in range(2)]
        inTf = [
            [
                const.tile([128, NLOC], BF16, tag=f"inTf{k}_{cc}", name=f"inTf{k}_{cc}")
                for cc in range(8)
            ]
            for k in range(2)
        ]
        w_sb = [const.tile([128, NHID], FP32, tag=f"w{k}", name=f"w{k}") for k in range(2)]
        wext = [const.tile([128, NHE], BF16, tag=f"wext{k}", name=f"wext{k}") for k in range(2)]
        wt_sb = const.tile([NHID, EMB], FP32)
        a_src = const.tile([NHID, 1], FP32, tag="asrc")
        a_dst = const.tile([NHID, 1], FP32, tag="adst")
        ident = const.tile([128, 128], FP32)
        identB = const.tile([128, 128], BF16, tag="identB")
        ones1 = const.tile([1, 128], BF16)
        wa_src = const.tile([128, 2], BF16, tag="wasrc")
        r_row = const.tile([1, NLOC], BF16)
        r_bc = const.tile([128, NLOC], BF16)
        neg1 = const.tile([128, 1], FP32, tag="neg1")
        sdst_st = const.tile([128, NT], FP32, tag="sdst")
        v_cols = const.tile([128, NT], FP32, tag="vcols")
        w_cols = const.tile([128, NT], FP32, tag="wcols")
        houT = const.tile([NHE, N], BF16, tag="houT")
        pay = const.tile([128, NT * NHE], BF16)
        outT = const.tile([NHE, NLOC], FP32, tag="outT")
        out_sb = const.tile([128, 8 * NHID], FP32)

        # preamble small DMAs on sync queue (ahead of the bulk streams)
        nc.sync.dma_start(wt_sb[:], WT_p[:])
        nc.sync.dma_start(a_src[:], a_p[0:NHID])
        nc.sync.dma_start(a_dst[:], a_p[NHID : 2 * NHID])
        for k in range(2):
            nc.scalar.dma_start(inTl[k][:], inTl_p[128 * k : 128 * (k + 1), :])
        for k in range(2):
            nc.scalar.dma_start(w_sb[k][:], W_p[128 * k : 128 * (k + 1), :])
        nc.scalar.dma_start(ident[:], ident_p[:])
        nc.scalar.dma_start(identB[:], identB_p[:])
        nc.vector.memset(ones1[:], 1.0)
        nc.vector.memset(neg1[:], -1.0)

        # ---- wa = W @ a_half for src and dst halves ----
        for half, asb in enumerate([a_src, a_dst]):
            for ec in range(2):
                ps = ps_scr.tile([128, 1], FP32, tag="scr", name=f"ps_wa{half}{ec}")
                nc.tensor.matmul(
                    ps[:], wt_sb[:, 128 * ec : 128 * (ec + 1)], asb[:],
                    start=True, stop=True,
                )
                if half == 0:
                    nc.scalar.copy(wa_src[:, ec : ec + 1], ps[:])
                else:
                    nc.scalar.copy(wext[ec][:, NHID : NHID + 1], ps[:])
        for ec in range(2):
            nc.scalar.copy(wext[ec][:, 0:NHID], w_sb[ec][:])

        # ---- s_src (local rows) row; r = exp((a-1)*s_src), broadcast ----
        for ih in range(2):
            ps = ps_scr.tile([1, 512], FP32, tag="scr", name=f"ps_ss{ih}")
            for kc in range(2):
                nc.tensor.matmul(
                    ps[:], wa_src[:, kc : kc + 1],
                    inTl[kc][:, 512 * ih : 512 * (ih + 1)],
                    start=(kc == 0), stop=(kc == 1),
                )
            nc.scalar.activation(
                r_row[:, 512 * ih : 512 * (ih + 1)], ps[:], ACTF.Exp,
                scale=ALPHA - 1.0,
            )
        for ih in range(2):
            ps = ps_scr.tile([128, 512], FP32, tag="scr", name=f"ps_rb{ih}")
            nc.tensor.matmul(
                ps[:], ones1[:], r_row[:, 512 * ih : 512 * (ih + 1)],
                start=True, stop=True,
            )
            nc.scalar.copy(r_bc[:, 512 * ih : 512 * (ih + 1)], ps[:])

        # ---- bulk DMA streams ----
        # sync queue: inTf chunks for groups 0-1 first, then interleave with
        # even adj groups; scalar queue: odd adj groups.
        def dma_inTf(cc):
            for k in range(2):
                nc.sync.dma_start(
                    inTf[k][cc][:],
                    inTf_p[128 * k : 128 * (k + 1), NLOC * cc : NLOC * (cc + 1)],
                )

        bfbufs, f8bufs = [], []
        for gr in range(NG):
            bfbufs.append(bf_pool.tile([128, N_BF * NLOC], BF16, tag="bfb", name=f"bfb{gr}"))
            if N_F8:
                f8bufs.append(f8_pool.tile([128, N_F8 * NLOC], FP8, tag="f8b", name=f"f8b{gr}"))

        dma_inTf(0)
        dma_inTf(1)
        for gr in range(NG):
            eng = nc.sync if gr % 2 == 0 else nc.scalar
            eng.dma_start(bfbufs[gr][:], adjB_p[gr])
            if N_F8:
                eng.dma_start(f8bufs[gr][:], adjF_p[gr])
            if gr < 6 and gr % 2 == 0:
                dma_inTf(2 + gr)
                dma_inTf(3 + gr)

        # ---- payload: h^T chunks -> transpose -> v-scaled payV ----
        pay3 = pay[:].rearrange("p (t e) -> p t e", e=NHE)
        for cc in range(8):
            hps = ps_h.tile([NHE, NLOC], FP32, tag="hps", name=f"hps{cc}")
            for kc in range(2):
                for ih in range(2):
                    nc.tensor.matmul(
                        hps[:, 512 * ih : 512 * (ih + 1)],
                        wext[kc][:],
                        inTf[kc][cc][:, 512 * ih : 512 * (ih + 1)],
                        start=(kc == 0), stop=(kc == 1),
                    )
            nc.scalar.copy(houT[:, NLOC * cc : NLOC * (cc + 1)], hps[:])
        PW = NHE + 1  # pad to 4-byte-aligned per-tile stride in PSUM
        for b in range(16):  # batches of 4 j-tiles
            pst = ps_t4.tile([128, 4 * PW], BF16, tag="t4", name=f"t4_{b}")
            for q in range(4):
                t = 4 * b + q
                nc.tensor.transpose(
                    pst[:, PW * q : PW * q + NHE],
                    houT[:, 128 * t : 128 * (t + 1)],
                    identB[:NHE, :NHE],
                )
            pst3 = pst[:].rearrange("p (q e) -> p q e", e=PW)
            nc.vector.tensor_copy(sdst_st[:, 4 * b : 4 * b + 4], pst3[:, :, NHID])
            nc.scalar.activation(
                v_cols[:, 4 * b : 4 * b + 4], sdst_st[:, 4 * b : 4 * b + 4], ACTF.Exp,
            )
            nc.scalar.activation(
                w_cols[:, 4 * b : 4 * b + 4], sdst_st[:, 4 * b : 4 * b + 4], ACTF.Exp,
                scale=ALPHA - 1.0,
            )
            for q in range(4):
                t = 4 * b + q
                nc.vector.tensor_scalar(
                    pay3[:, t, 0:NHID], pst3[:, q, 0:NHID],
                    v_cols[:, t : t + 1], None, ALU.mult,
                )
            # denominator column = v itself
            nc.vector.tensor_copy(pay3[:, 4 * b : 4 * b + 4, NHID], v_cols[:, 4 * b : 4 * b + 4])

        # ---- main loop over 8 groups ----
        out_acc = ps_acc.tile([NHE, NLOC], FP32, tag="acc")
        mm_started = [False, False]
        # last matmul per half: group 7 slot 7 (g-form)
        for gr in range(NG):
            kapb = kap_pool.tile([128, N_BF * NLOC], BF16, tag="kapb", name=f"kap{gr}")
            gkap = gkap_pool.tile([128, N_F8 * NLOC], BF16, tag="gkap", name=f"gkap{gr}") if N_F8 else None
            p5 = p_pool.tile([128, N_BF * NLOC], BF16, tag="p5", name=f"p5_{gr}")
            pg = pg_pool.tile([128, N_F8 * NLOC], BF16, tag="pg", name=f"pg{gr}") if N_F8 else None
            for slot in range(N_BF):
                t = gr * 8 + slot
                sl = slice(NLOC * slot, NLOC * (slot + 1))
                if FORM[(gr, slot)] == "y":
                    nc.scalar.activation(
                        kapb[:, sl], r_bc[:], ACTF.Relu,
                        bias=neg1[:, 0:1], scale=w_cols[:, t : t + 1],
                    )
                else:
                    nc.vector.tensor_scalar(
                        kapb[:, sl], r_bc[:], w_cols[:, t : t + 1], 1.0,
                        ALU.mult, ALU.max,
                    )
            nc.vector.tensor_mul(p5[:], kapb[:], bfbufs[gr][:])
            for slot in range(N_BF, 8):
                t = gr * 8 + slot
                sl = slice(NLOC * (slot - N_BF), NLOC * (slot - N_BF + 1))
                nc.vector.tensor_scalar(
                    gkap[:, sl], r_bc[:], w_cols[:, t : t + 1], 1.0,
                    ALU.mult, ALU.max,
                )
                nc.gpsimd.tensor_tensor(pg[:, sl], gkap[:, sl], f8bufs[gr][:, sl], ALU.mult)
            # matmuls
            for slot in range(8):
                t = gr * 8 + slot
                form = FORM[(gr, slot)]
                if form == "g":
                    src, off = pg, NLOC * (slot - N_BF)
                else:
                    src, off = p5, NLOC * slot
                movings = [src[:, off + 512 * ih : off + 512 * (ih + 1)] for ih in range(2)]
                extra = []
                if form == "y":
                    extra = [
                        bfbufs[gr][:, NLOC * slot + 512 * ih : NLOC * slot + 512 * (ih + 1)]
                        for ih in range(2)
                    ]
                for ih in range(2):
                    is_last = gr == NG - 1 and slot == 7
                    nc.tensor.matmul(
                        out_acc[:, 512 * ih : 512 * (ih + 1)],
                        pay3[:, t, 0:NHE],
                        movings[ih],
                        start=not mm_started[ih], stop=is_last,
                    )
                    mm_started[ih] = True
                    if extra:
                        nc.tensor.matmul(
                            out_acc[:, 512 * ih : 512 * (ih + 1)],
                            pay3[:, t, 0:NHE],
                            extra[ih],
                            start=False, stop=False,
                        )

        # ---- normalize + transpose + store ----
        nc.scalar.copy(outT[:], out_acc[:])
        for ic in range(8):
            ps_t = ps_scr.tile([128, NHE], FP32, tag="scr", name=f"ps_o{ic}")
            nc.tensor.transpose(
                ps_t[:], outT[:, 128 * ic : 128 * (ic + 1)], ident[:NHE, :NHE]
            )
            zrec = small.tile([128, 1], FP32, tag="zrec", name=f"zrec{ic}")
            nc.vector.reciprocal(zrec[:], ps_t[:, NHID : NHID + 1])
            nc.vector.tensor_scalar(
                out_sb[:, NHID * ic : NHID * (ic + 1)], ps_t[:, 0:NHID],
                zrec[:], None, ALU.mult,
            )
        nc.sync.dma_start(out_p[:], out_sb[:])

    return nc


def shard_inputs(input, adj, W, a):
    """Host-side sharding/layout prep. Returns in_maps for the 8 cores."""
    input = np.asarray(input, dtype=np.float32)
    adj = np.asarray(adj, dtype=np.int32)
    W = np.ascontiguousarray(np.asarray(W, dtype=np.float32))
    a = np.ascontiguousarray(np.asarray(a, dtype=np.float32))
    inputT = np.ascontiguousarray(input.T.astype(ml_dtypes.bfloat16))
    adjT = adj.T  # [j, i] int32
    WT = np.ascontiguousarray(W.T)
    ident = np.eye(128, dtype=np.float32)
    identB = np.eye(128, dtype=ml_dtypes.bfloat16)
    in_maps = []
    for c in range(NCORES):
        rows = slice(c * NLOC, (c + 1) * NLOC)
        A = adjT[:, rows].reshape(NG, 8, 128, NLOC)  # [gr, slot, p, i]
        adjB = np.ascontiguousarray(
            A[:, 0:N_BF].transpose(0, 2, 1, 3).reshape(NG, 128, N_BF * NLOC)
        ).astype(ml_dtypes.bfloat16)
        adjF = (np.ascontiguousarray(
            A[:, N_BF:8].transpose(0, 2, 1, 3).reshape(NG, 128, N_F8 * NLOC)
        ).astype(ml_dtypes.float8_e4m3fn) if N_F8 else None)
        in_maps.append(
            {
                "adjB": adjB,
                **({"adjF": adjF} if N_F8 else {}),
                "inTl": np.ascontiguousarray(inputT[:, rows]),
                "inTf": inputT,
                "W": W,
                "WT": WT,
                "a": a,
                "ident": ident,
                "identB": identB,
            }
        )
    return in_maps


_CACHE = {}


def kernel(input, adj, W, a, _trace=False, _return_result=False):
    from concourse.bass_utils import run_bass_kernel_spmd

    if "nc" not in _CACHE:
        _CACHE["nc"] = build_kernel()
    nc = _CACHE["nc"]
    in_maps = shard_inputs(input, adj, W, a)
    res = run_bass_kernel_spmd(
        nc, in_maps, core_ids=list(range(NCORES)), trace=_trace
    )
    out = np.concatenate(
        [
            res.results[c]["out"]
            .reshape(128, 8, NHID)
            .transpose(1, 0, 2)
            .reshape(NLOC, NHID)
            for c in range(NCORES)
        ],
        axis=0,
    )
    if _return_result:
        return out, res
    return out


if __name__ == "__main__":
    rng = np.random.default_rng(0)
    inp = rng.standard_normal((N, EMB), dtype=np.float32)
    adj = rng.integers(0, 2, size=(N, N), dtype=np.int32)
    W = (rng.standard_normal((EMB, NHID)) * 0.05).astype(np.float32)
    a = (rng.standard_normal(2 * NHID) * 0.05).astype(np.float32)
    out = kernel(inp, adj, W, a)
    print(out.shape, out.dtype)
